# revision 2
# baseline (speedup 1.0000x reference)
"""Trainium2 Bass kernel for nn_Attention_6932077215914 (GQA attention layer).

Strategy (8 NeuronCores, tensor-parallel over heads + sequence-parallel dense):
  - Host: x -> x^T (bf16), w_q * softmax_scale, shard w_q/w_kv by head/group,
    full w_dense (bf16) everywhere.
  - Core c owns heads {2c, 2c+1} (KV group c//2, K/V proj duplicated per pair).
  - QKV projections from x^T produce Q^T/K^T/V^T in [e, t] layout; V^T is
    XBAR-transposed to V[k, d]. Scores computed [q, k] (causally trimmed),
    exp on ScalarE with accumulated row sums, normalized in place on DVE,
    XBAR-transposed to E^T[k, q] for the PV matmul -> ctx^T[d, q].
  - AllToAll (4 chunks: batch x local-head) redistributes ctx^T so each core
    owns a 256-token slice of the sequence for the dense projection, which
    then produces the final [t, o] rows directly (no output transpose).
"""

import sys
import types

import numpy as np
import ml_dtypes

B, SQ, HIDDEN = 2, 2048, 2048
HEADS, GROUPS, KVC = 16, 4, 128
SCALE = KVC ** -0.5
NCORES = 8
T = B * SQ               # 4096 flattened tokens
TC = 512                 # t-chunk for QKV projection
NTC = T // TC            # 8
NQT = SQ // 128          # 16 q-tiles per batch
MASK_VAL = -1e30


def _install_ntff_hook():
    """boot() skips NTFF hook registration when the image's antenv lacks
    axon_hooks; recreate the tiny module so trace=True / BASS_TRACE works."""
    if "antenv.axon_hooks" in sys.modules:
        return
    try:
        from trn_agent_boot.trn_boot import _ntff_profile_via_ctypes
        hook = _ntff_profile_via_ctypes("/opt/axon/libaxon_pjrt.so")
    except Exception:
        return
    mod = types.ModuleType("antenv.axon_hooks")
    mod.get_axon_ntff_profile_hook = lambda: hook
    mod.set_axon_ntff_profile_hook = lambda h: None
    sys.modules["antenv.axon_hooks"] = mod


_install_ntff_hook()

_CACHE = {}


def _build():
    import concourse.bass as bass
    import concourse.mybir as mybir
    import concourse.tile as tile
    from concourse import bacc
    from concourse.bass import ts, ds

    BF16 = mybir.dt.bfloat16
    F32 = mybir.dt.float32
    AF = mybir.ActivationFunctionType

    nc = bacc.Bacc("TRN2", target_bir_lowering=False, debug=False,
                   num_devices=NCORES)

    xt = nc.dram_tensor("xt", [HIDDEN, T], BF16, kind="ExternalInput")
    wq = nc.dram_tensor("wq", [HIDDEN, 256], BF16, kind="ExternalInput")
    wk = nc.dram_tensor("wk", [HIDDEN, 128], BF16, kind="ExternalInput")
    wv = nc.dram_tensor("wv", [HIDDEN, 128], BF16, kind="ExternalInput")
    wd = nc.dram_tensor("wd", [HIDDEN, HIDDEN], BF16, kind="ExternalInput")
    out = nc.dram_tensor("out", [512, HIDDEN], F32, kind="ExternalOutput")

    with tile.TileContext(nc) as tc:
        import contextlib
        with contextlib.ExitStack() as ctx:
            # ---- long-lived pools ----
            persist = ctx.enter_context(tc.tile_pool(name="persist", bufs=1))
            dram = ctx.enter_context(tc.tile_pool(name="dram", bufs=1, space="DRAM"))

            # causal masks for the diagonal 512-wide k-chunk, r = qt % 4:
            # mask_r[p, f] = 0 if f <= 128*r + p else MASK_VAL
            masks = persist.tile([128, 4, 512], F32, name="masks")
            for r in range(4):
                m = masks[:, r, :]
                nc.gpsimd.memset(m, 0.0)
                nc.gpsimd.affine_select(
                    out=m, in_=m,
                    compare_op=mybir.AluOpType.is_ge,
                    fill=MASK_VAL, base=128 * r,
                    pattern=[[-1, 512]], channel_multiplier=1,
                )

            # weight SBUF tiles: [p, h-chunk, e]
            wq_sb = persist.tile([128, 16, 256], BF16, name="wq_sb")
            nc.sync.dma_start(out=wq_sb[:], in_=wq.ap().rearrange("(c p) e -> p c e", p=128))
            wk_sb = persist.tile([128, 16, 128], BF16, name="wk_sb")
            nc.sync.dma_start(out=wk_sb[:], in_=wk.ap().rearrange("(c p) e -> p c e", p=128))
            wv_sb = persist.tile([128, 16, 128], BF16, name="wv_sb")
            nc.sync.dma_start(out=wv_sb[:], in_=wv.ap().rearrange("(c p) e -> p c e", p=128))

            # resident Q^T / K^T / V per batch (bf16)
            q_res = [[persist.tile([128, SQ], BF16, name=f"q{h}{b}")
                      for b in range(B)] for h in range(2)]
            k_res = [persist.tile([128, SQ], BF16, name=f"k{b}") for b in range(B)]
            v_res = [persist.tile([128, 16, 128], BF16, name=f"v{b}") for b in range(B)]

            # A2A bounce buffers, one per (b, h_local) chunk
            cc_in = [[dram.tile([NCORES * 128, 256], BF16, name=f"ccin{b}{h}")
                      for h in range(2)] for b in range(B)]
            cc_out = [[dram.tile([NCORES, 128, 256], BF16, name=f"ccout{b}{h}")
                       for h in range(2)] for b in range(B)]

            # dense weight tile [p, e-chunk, o] (8.4 MB) - loaded during attention
            wd_sb = persist.tile([128, 16, HIDDEN], BF16, name="wd_sb")

            # ---- phase 1: QKV projections ----
            with tc.tile_pool(name="p1", bufs=2) as p1, \
                 tc.tile_pool(name="p1ps", bufs=2, space="PSUM") as p1ps:
                for tci in range(NTC):
                    b, sc = tci // 4, tci % 4
                    xt_sb = p1.tile([128, 16, TC], BF16, tag="xt")
                    nc.sync.dma_start(
                        out=xt_sb[:],
                        in_=xt.ap()[:, ts(tci, TC)].rearrange("(c p) t -> p c t", p=128),
                    )
                    outs = []
                    for name, w_ap in (("q0", wq_sb[:, :, 0:128]),
                                       ("q1", wq_sb[:, :, 128:256]),
                                       ("kk", wk_sb[:, :, :]),
                                       ("vv", wv_sb[:, :, :])):
                        ps = p1ps.tile([128, TC], F32, tag=name)
                        for hc in range(16):
                            nc.tensor.matmul(ps[:], w_ap[:, hc, :], xt_sb[:, hc, :],
                                             start=(hc == 0), stop=(hc == 15))
                        outs.append(ps)
                    nc.vector.tensor_copy(q_res[0][b][:, ts(sc, TC)], outs[0][:])
                    nc.vector.tensor_copy(q_res[1][b][:, ts(sc, TC)], outs[1][:])
                    nc.vector.tensor_copy(k_res[b][:, ts(sc, TC)], outs[2][:])
                    vt_sb = p1.tile([128, TC], BF16, tag="vt")
                    nc.vector.tensor_copy(vt_sb[:], outs[3][:])
                    # V^T [d, s] -> V [s, d] tiles via XBAR
                    nc.sync.dma_start(out=v_res[b][:, ds(4 * sc, 4), :],
                                      in_=vt_sb[:], transpose=True)

            # prefetch dense weights (DMA is idle during attention)
            nc.sync.dma_start(out=wd_sb[:],
                              in_=wd.ap().rearrange("(c p) o -> p c o", p=128))

            # ---- phase 2: attention per (b, h_local) ----
            with tc.tile_pool(name="p2", bufs=2) as p2, \
                 tc.tile_pool(name="p2s", bufs=4) as p2s, \
                 tc.tile_pool(name="p2ps", bufs=3, space="PSUM") as p2ps:
                for b in range(B):
                    for hl in range(2):
                        for qg in range(4):
                            nch = qg + 1          # 512-wide k-chunks per q-tile
                            nkt = 4 * nch         # 128-wide k-tiles for PV
                            et_slab = p2.tile([128, 16, 512], BF16, tag="et")
                            for qt4 in range(4):
                                qt = qg * 4 + qt4
                                e_row = p2.tile([128, SQ], BF16, tag="erow")
                                rs_parts = p2s.tile([128, 4], F32, tag="rsp")
                                for j in range(nch):
                                    sc_ps = p2ps.tile([128, 512], F32, tag="sc")
                                    nc.tensor.matmul(
                                        sc_ps[:],
                                        q_res[hl][b][:, ts(qt, 128)],
                                        k_res[b][:, ts(j, 512)],
                                        start=True, stop=True)
                                    if j == qg:  # diagonal chunk: causal mask
                                        nc.vector.tensor_add(
                                            sc_ps[:], sc_ps[:], masks[:, qt4, :])
                                    nc.scalar.activation(
                                        e_row[:, ts(j, 512)], sc_ps[:], AF.Exp,
                                        accum_out=rs_parts[:, j:j + 1])
                                rinv = p2s.tile([128, 1], F32, tag="rinv")
                                if nch > 1:
                                    rsum = p2s.tile([128, 1], F32, tag="rsum")
                                    nc.vector.reduce_sum(
                                        rsum[:], rs_parts[:, 0:nch],
                                        axis=mybir.AxisListType.X)
                                    nc.vector.reciprocal(rinv[:], rsum[:])
                                else:
                                    nc.vector.reciprocal(rinv[:], rs_parts[:, 0:1])
                                nc.vector.tensor_scalar_mul(
                                    e_row[:, 0:512 * nch], e_row[:, 0:512 * nch],
                                    rinv[:])
                                for j in range(nch):
                                    nc.sync.dma_start(
                                        out=et_slab[:, ds(4 * j, 4), ts(qt4, 128)],
                                        in_=e_row[:, ts(j, 512)], transpose=True)
                            # PV: ctx^T[d, q] over k-tiles
                            ctx_ps = p2ps.tile([128, 512], F32, tag="ctx")
                            for kt in range(nkt):
                                nc.tensor.matmul(ctx_ps[:], v_res[b][:, kt, :],
                                                 et_slab[:, kt, :],
                                                 start=(kt == 0), stop=(kt == nkt - 1))
                            ctxt_sb = p2.tile([128, 512], BF16, tag="ctxt")
                            nc.vector.tensor_copy(ctxt_sb[:], ctx_ps[:])
                            # scatter to A2A bounce: q-group qg covers peers 2qg, 2qg+1
                            for half in range(2):
                                peer = 2 * qg + half
                                nc.sync.dma_start(
                                    out=cc_in[b][hl][ts(peer, 128), :],
                                    in_=ctxt_sb[:, ts(half, 256)])
                        nc.gpsimd.collective_compute(
                            "AllToAll", mybir.AluOpType.bypass,
                            replica_groups=[list(range(NCORES))],
                            ins=[cc_in[b][hl].opt()],
                            outs=[cc_out[b][hl].opt()])

            # ---- phase 3: dense projection on my 256-token slice per batch ----
            with tc.tile_pool(name="p3", bufs=2) as p3, \
                 tc.tile_pool(name="p3g", bufs=2) as p3g, \
                 tc.tile_pool(name="p3ps", bufs=2, space="PSUM") as p3ps:
                for b in range(B):
                    g_all = []
                    for hl in range(2):
                        g = p3g.tile([128, NCORES, 256], BF16, tag=f"g{hl}")
                        nc.sync.dma_start(
                            out=g[:],
                            in_=cc_out[b][hl].rearrange("i p s -> p i s"))
                        g_all.append(g)
                    for u in range(2):
                        o_ps = p3ps.tile([128, HIDDEN], F32, tag="ops")
                        for ec in range(16):
                            i, hl = ec // 2, ec % 2
                            lhsT = g_all[hl][:, i, ts(u, 128)]
                            for oc in range(4):
                                nc.tensor.matmul(
                                    o_ps[:, ts(oc, 512)], lhsT,
                                    wd_sb[:, 2 * i + hl, ts(oc, 512)],
                                    start=(ec == 0), stop=(ec == 15))
                        o_sb = p3.tile([128, HIDDEN], F32, tag="osb")
                        nc.vector.tensor_copy(o_sb[:], o_ps[:])
                        nc.sync.dma_start(
                            out=out.ap()[ds(b * 256 + u * 128, 128), :],
                            in_=o_sb[:])

    nc.compile()
    return nc


def kernel(x, w_q, w_kv, w_dense):
    from concourse.bass_utils import run_bass_kernel_spmd

    bf16 = ml_dtypes.bfloat16
    x = np.asarray(x, dtype=np.float32)
    w_q = np.asarray(w_q, dtype=np.float32)
    w_kv = np.asarray(w_kv, dtype=np.float32)
    w_dense = np.asarray(w_dense, dtype=np.float32)

    xt = np.ascontiguousarray(x.reshape(T, HIDDEN).T).astype(bf16)
    wq_s = (w_q * SCALE).astype(bf16)          # fold softmax scale into Q proj
    wkv_b = w_kv.astype(bf16)
    wd_b = w_dense.astype(bf16)

    in_maps = []
    for c in range(NCORES):
        g = c // 2
        in_maps.append({
            "xt": xt,
            "wq": np.ascontiguousarray(wq_s[:, 256 * c:256 * (c + 1)]),
            "wk": np.ascontiguousarray(wkv_b[:, 128 * g:128 * (g + 1)]),
            "wv": np.ascontiguousarray(wkv_b[:, 512 + 128 * g:512 + 128 * (g + 1)]),
            "wd": wd_b,
        })

    if "nc" not in _CACHE:
        _CACHE["nc"] = _build()
    nc = _CACHE["nc"]

    res = run_bass_kernel_spmd(nc, in_maps, core_ids=list(range(NCORES)))
    kernel.last_results = res
    kernel.last_exec_time_ns = res.exec_time_ns

    out_full = np.empty((T, HIDDEN), dtype=np.float32)
    for c in range(NCORES):
        r = res.results[c]["out"]              # [512, 2048]
        for b in range(B):
            out_full[b * SQ + 256 * c: b * SQ + 256 * (c + 1), :] = \
                r[b * 256:(b + 1) * 256, :]
    return out_full.reshape(B, SQ, HIDDEN)


# revision 7
# speedup vs baseline: 1.2799x; 1.2799x over previous
"""Trainium2 Bass kernel for nn_Attention_6932077215914 (GQA attention layer).

Strategy (8 NeuronCores, tensor-parallel over heads + sequence-parallel dense):
  - Host: x -> x^T (bf16), w_q * softmax_scale, shard w_q/w_kv by head/group,
    full w_dense (bf16) everywhere.
  - Core c owns heads {2c, 2c+1} (KV group c//2, K/V proj duplicated per pair).
  - QKV projections from x^T produce Q^T/K^T/V^T in [e, t] layout; V^T is
    XBAR-transposed to V[k, d]. Scores computed [q, k] (causally trimmed),
    exp on ScalarE with accumulated row sums, normalized in place on DVE,
    XBAR-transposed to E^T[k, q] for the PV matmul -> ctx^T[d, q].
  - AllToAll (4 chunks: batch x local-head) redistributes ctx^T so each core
    owns a 256-token slice of the sequence for the dense projection, which
    then produces the final [t, o] rows directly (no output transpose).
"""

import sys
import types

import numpy as np
import ml_dtypes

B, SQ, HIDDEN = 2, 2048, 2048
HEADS, GROUPS, KVC = 16, 4, 128
SCALE = KVC ** -0.5
NCORES = 8
T = B * SQ               # 4096 flattened tokens
TC = 512                 # t-chunk for QKV projection
NTC = T // TC            # 8
NQT = SQ // 128          # 16 q-tiles per batch
MASK_VAL = -1e30


def _install_ntff_hook():
    """boot() skips NTFF hook registration when the image's antenv lacks
    axon_hooks; recreate the tiny module so trace=True / BASS_TRACE works."""
    if "antenv.axon_hooks" in sys.modules:
        return
    try:
        from trn_agent_boot.trn_boot import _ntff_profile_via_ctypes
        hook = _ntff_profile_via_ctypes("/opt/axon/libaxon_pjrt.so")
    except Exception:
        return
    mod = types.ModuleType("antenv.axon_hooks")
    mod.get_axon_ntff_profile_hook = lambda: hook
    mod.set_axon_ntff_profile_hook = lambda h: None
    sys.modules["antenv.axon_hooks"] = mod


_install_ntff_hook()

_CACHE = {}


def _build():
    import concourse.bass as bass
    import concourse.mybir as mybir
    import concourse.tile as tile
    from concourse import bacc
    from concourse.bass import ts, ds

    BF16 = mybir.dt.bfloat16
    F32 = mybir.dt.float32
    AF = mybir.ActivationFunctionType

    nc = bacc.Bacc("TRN2", target_bir_lowering=False, debug=False,
                   num_devices=NCORES)

    xt = nc.dram_tensor("xt", [HIDDEN, T], BF16, kind="ExternalInput")
    wq = nc.dram_tensor("wq", [HIDDEN, 256], BF16, kind="ExternalInput")
    wk = nc.dram_tensor("wk", [HIDDEN, 128], BF16, kind="ExternalInput")
    wv = nc.dram_tensor("wv", [HIDDEN, 128], BF16, kind="ExternalInput")
    wd = nc.dram_tensor("wd", [HIDDEN, HIDDEN], BF16, kind="ExternalInput")
    out = nc.dram_tensor("out", [512, HIDDEN], F32, kind="ExternalOutput")

    with tile.TileContext(nc) as tc:
        import contextlib
        with contextlib.ExitStack() as ctx:
            # ---- long-lived pools ----
            persist = ctx.enter_context(tc.tile_pool(name="persist", bufs=1))
            dram = ctx.enter_context(tc.tile_pool(name="dram", bufs=1, space="DRAM"))

            # causal masks in scores^T [k, q] orientation for the diagonal
            # k-tile, r = kt - 4*qg: keep (0) when f >= 128*r + p
            masks = persist.tile([128, 4, 512], F32, name="masks")
            for r in range(4):
                m = masks[:, r, :]
                nc.gpsimd.memset(m, 0.0)
                nc.gpsimd.affine_select(
                    out=m, in_=m,
                    compare_op=mybir.AluOpType.is_ge,
                    fill=MASK_VAL, base=-128 * r,
                    pattern=[[1, 512]], channel_multiplier=-1,
                )
            ones_col = persist.tile([128, 1], BF16, name="ones_col")
            nc.vector.memset(ones_col[:], 1.0)
            ones_row = persist.tile([1, 128], F32, name="ones_row")
            nc.vector.memset(ones_row[:], 1.0)

            # weight SBUF tiles: [p, h-chunk, e]
            wq_sb = persist.tile([128, 16, 256], BF16, name="wq_sb")
            nc.sync.dma_start(out=wq_sb[:], in_=wq.ap().rearrange("(c p) e -> p c e", p=128))
            wk_sb = persist.tile([128, 16, 128], BF16, name="wk_sb")
            nc.sync.dma_start(out=wk_sb[:], in_=wk.ap().rearrange("(c p) e -> p c e", p=128))
            wv_sb = persist.tile([128, 16, 128], BF16, name="wv_sb")
            nc.sync.dma_start(out=wv_sb[:], in_=wv.ap().rearrange("(c p) e -> p c e", p=128))

            # resident Q^T / K^T / V per batch (bf16)
            q_res = [[persist.tile([128, SQ], BF16, name=f"q{h}{b}")
                      for b in range(B)] for h in range(2)]
            k_res = [persist.tile([128, SQ], BF16, name=f"k{b}") for b in range(B)]
            v_res = [persist.tile([128, 16, 128], BF16, name=f"v{b}") for b in range(B)]

            # A2A bounce buffers, one per (b, h_local) chunk
            cc_in = [[dram.tile([NCORES * 128, 256], BF16, name=f"ccin{b}{h}")
                      for h in range(2)] for b in range(B)]
            cc_out = [[dram.tile([NCORES, 128, 256], BF16, name=f"ccout{b}{h}")
                       for h in range(2)] for b in range(B)]

            # dense weight tile [p, e-chunk, o] (8.4 MB) - loaded during attention
            wd_sb = persist.tile([128, 16, HIDDEN], BF16, name="wd_sb")

            # ---- phase 1: QKV projections ----
            with tc.tile_pool(name="p1", bufs=2) as p1, \
                 tc.tile_pool(name="p1ps", bufs=2, space="PSUM") as p1ps:
                for tci in range(NTC):
                    b, sc = tci // 4, tci % 4
                    xt_sb = p1.tile([128, 16, TC], BF16, tag="xt")
                    nc.sync.dma_start(
                        out=xt_sb[:],
                        in_=xt.ap()[:, ts(tci, TC)].rearrange("(c p) t -> p c t", p=128),
                    )
                    outs = []
                    for name, w_ap in (("q0", wq_sb[:, :, 0:128]),
                                       ("q1", wq_sb[:, :, 128:256]),
                                       ("kk", wk_sb[:, :, :]),
                                       ("vv", wv_sb[:, :, :])):
                        ps = p1ps.tile([128, TC], F32, tag=name)
                        for hc in range(16):
                            nc.tensor.matmul(ps[:], w_ap[:, hc, :], xt_sb[:, hc, :],
                                             start=(hc == 0), stop=(hc == 15))
                        outs.append(ps)
                    nc.vector.tensor_copy(q_res[0][b][:, ts(sc, TC)], outs[0][:])
                    nc.vector.tensor_copy(q_res[1][b][:, ts(sc, TC)], outs[1][:])
                    nc.vector.tensor_copy(k_res[b][:, ts(sc, TC)], outs[2][:])
                    vt_sb = p1.tile([128, TC], BF16, tag="vt")
                    nc.vector.tensor_copy(vt_sb[:], outs[3][:])
                    # V^T [d, s] -> V [s, d] tiles via XBAR
                    nc.sync.dma_start(out=v_res[b][:, ds(4 * sc, 4), :],
                                      in_=vt_sb[:], transpose=True)

            # prefetch dense weights (DMA is idle during attention)
            nc.sync.dma_start(out=wd_sb[:],
                              in_=wd.ap().rearrange("(c p) o -> p c o", p=128))

            # ---- phase 2: attention per (b, h_local), scores^T [k, q] ----
            with tc.tile_pool(name="p2", bufs=2) as p2, \
                 tc.tile_pool(name="p2s", bufs=4) as p2s, \
                 tc.tile_pool(name="p2ps", bufs=2, space="PSUM") as p2ps, \
                 tc.tile_pool(name="p2sc", bufs=3, space="PSUM") as p2sc:
                for b in range(B):
                    for hl in range(2):
                        for qg in range(4):
                            nkt = 4 * (qg + 1)    # causal 128-wide k-tiles
                            et_slab = p2.tile([128, 16, 512], BF16, tag="et")
                            # QK^T -> mask -> exp, writes E^T[k, q] directly
                            for kt in range(nkt):
                                sc_ps = p2sc.tile([128, 512], F32, tag="sc")
                                nc.tensor.matmul(
                                    sc_ps[:],
                                    k_res[b][:, ts(kt, 128)],
                                    q_res[hl][b][:, ts(qg, 512)],
                                    start=True, stop=True)
                                r = kt - 4 * qg
                                if r >= 0:        # diagonal k-tile: causal mask
                                    nc.vector.tensor_add(
                                        sc_ps[:], sc_ps[:], masks[:, r, :])
                                nc.scalar.activation(
                                    et_slab[:, kt, :], sc_ps[:], AF.Exp)
                            # row sums via ones-matmuls + PV, both over k-tiles
                            rs_ps = p2ps.tile([1, 512], F32, tag="rs", bufs=1)
                            ctx_ps = p2ps.tile([128, 512], F32, tag="ctx")
                            for kt in range(nkt):
                                nc.tensor.matmul(rs_ps[:], ones_col[:],
                                                 et_slab[:, kt, :],
                                                 start=(kt == 0), stop=(kt == nkt - 1))
                                nc.tensor.matmul(ctx_ps[:], v_res[b][:, kt, :],
                                                 et_slab[:, kt, :],
                                                 start=(kt == 0), stop=(kt == nkt - 1))
                            # 1/rowsum broadcast to 128 partitions via PE
                            rinv_sb = p2s.tile([1, 512], F32, tag="rinv")
                            nc.vector.reciprocal(rinv_sb[:], rs_ps[:])
                            bc_ps = p2ps.tile([128, 512], F32, tag="bc", bufs=1)
                            nc.tensor.matmul(bc_ps[:], ones_row[:], rinv_sb[:],
                                             start=True, stop=True)
                            bc_sb = p2s.tile([128, 512], F32, tag="bcs", bufs=2)
                            nc.vector.tensor_copy(bc_sb[:], bc_ps[:])
                            # normalized ctx^T -> bf16
                            ctxt_sb = p2.tile([128, 512], BF16, tag="ctxt")
                            nc.vector.tensor_mul(ctxt_sb[:], ctx_ps[:], bc_sb[:])
                            # scatter to A2A bounce: q-group qg covers peers 2qg, 2qg+1
                            for half in range(2):
                                peer = 2 * qg + half
                                nc.sync.dma_start(
                                    out=cc_in[b][hl][ts(peer, 128), :],
                                    in_=ctxt_sb[:, ts(half, 256)])
                        nc.gpsimd.collective_compute(
                            "AllToAll", mybir.AluOpType.bypass,
                            replica_groups=[list(range(NCORES))],
                            ins=[cc_in[b][hl].opt()],
                            outs=[cc_out[b][hl].opt()])

            # ---- phase 3: dense projection on my 256-token slice per batch ----
            with tc.tile_pool(name="p3", bufs=2) as p3, \
                 tc.tile_pool(name="p3g", bufs=2) as p3g, \
                 tc.tile_pool(name="p3ps", bufs=2, space="PSUM") as p3ps:
                for b in range(B):
                    g_all = []
                    for hl in range(2):
                        g = p3g.tile([128, NCORES, 256], BF16, tag=f"g{hl}")
                        nc.sync.dma_start(
                            out=g[:],
                            in_=cc_out[b][hl].rearrange("i p s -> p i s"))
                        g_all.append(g)
                    for u in range(2):
                        o_ps = p3ps.tile([128, HIDDEN], F32, tag="ops")
                        for ec in range(16):
                            i, hl = ec // 2, ec % 2
                            lhsT = g_all[hl][:, i, ts(u, 128)]
                            for oc in range(4):
                                nc.tensor.matmul(
                                    o_ps[:, ts(oc, 512)], lhsT,
                                    wd_sb[:, 2 * i + hl, ts(oc, 512)],
                                    start=(ec == 0), stop=(ec == 15))
                        o_sb = p3.tile([128, HIDDEN], F32, tag="osb")
                        nc.vector.tensor_copy(o_sb[:], o_ps[:])
                        nc.sync.dma_start(
                            out=out.ap()[ds(b * 256 + u * 128, 128), :],
                            in_=o_sb[:])

    nc.compile()
    return nc


def kernel(x, w_q, w_kv, w_dense):
    from concourse.bass_utils import run_bass_kernel_spmd

    bf16 = ml_dtypes.bfloat16
    x = np.asarray(x, dtype=np.float32)
    w_q = np.asarray(w_q, dtype=np.float32)
    w_kv = np.asarray(w_kv, dtype=np.float32)
    w_dense = np.asarray(w_dense, dtype=np.float32)

    xt = np.ascontiguousarray(x.reshape(T, HIDDEN).T).astype(bf16)
    wq_s = (w_q * SCALE).astype(bf16)          # fold softmax scale into Q proj
    wkv_b = w_kv.astype(bf16)
    wd_b = w_dense.astype(bf16)

    in_maps = []
    for c in range(NCORES):
        g = c // 2
        in_maps.append({
            "xt": xt,
            "wq": np.ascontiguousarray(wq_s[:, 256 * c:256 * (c + 1)]),
            "wk": np.ascontiguousarray(wkv_b[:, 128 * g:128 * (g + 1)]),
            "wv": np.ascontiguousarray(wkv_b[:, 512 + 128 * g:512 + 128 * (g + 1)]),
            "wd": wd_b,
        })

    if "nc" not in _CACHE:
        _CACHE["nc"] = _build()
    nc = _CACHE["nc"]

    res = run_bass_kernel_spmd(nc, in_maps, core_ids=list(range(NCORES)))
    kernel.last_results = res
    kernel.last_exec_time_ns = res.exec_time_ns

    out_full = np.empty((T, HIDDEN), dtype=np.float32)
    for c in range(NCORES):
        r = res.results[c]["out"]              # [512, 2048]
        for b in range(B):
            out_full[b * SQ + 256 * c: b * SQ + 256 * (c + 1), :] = \
                r[b * 256:(b + 1) * 256, :]
    return out_full.reshape(B, SQ, HIDDEN)


# revision 71
# speedup vs baseline: 1.8260x; 1.4266x over previous
"""Trainium2 Bass kernel for nn_Attention_6932077215914 (GQA attention layer).

Strategy (8 NeuronCores, tensor-parallel over heads + sequence-parallel dense):
  - Host prep: x -> x^T (bf16, transposed so the hidden contraction dim lands
    on SBUF partitions), softmax scale folded into w_q, w_q/w_kv sharded by
    head/KV-group, all weights pre-tiled to their SBUF layout for multi-KB
    contiguous DMA runs. Compute is bf16 with fp32 PSUM accumulation.
  - Core c owns heads {2c, 2c+1} (KV group c//2). Within each core pair the
    KV projection is split: even cores compute K^T, odd cores V^T (the split
    lives in the per-core weight data, so the graph stays SPMD-uniform), and
    a per-batch 2-rank AllGather exchanges the halves, fully hidden under the
    remaining Q-projection matmuls.
  - Phase 1: Q/KV projections from resident x^T produce Q^T and K^T-or-V^T in
    [e, t] layout; V^T is XBAR-DMA-transposed into V[k, d] tiles on the way
    back from the exchange.
  - Phase 2 (per batch x local head): scores are computed TRANSPOSED,
    S^T[k, q] = K^T.T @ Q^T, causally trimmed at 512x128 granularity with the
    diagonal tiles narrowed to their valid q-suffix; exp on ScalarE writes
    E^T[k, q] straight into the PV operand layout (no transpose anywhere);
    GpSimd affine_select zeroes the q < k triangle. Row sums: 1/4 of k-tiles
    folded on PE via ones-matmuls into PSUM, the rest summed on DVE and folded
    by one f32 matmul; the sums are partition-broadcast by a PE rank-1 matmul
    and inverted with reciprocal_approx_fast. PV accumulates ctx^T[d, q],
    normalized during the PSUM->SBUF epilogue.
  - AllToAll (4 chunks: batch x local-head, overlapped with attention)
    redistributes ctx^T so each core owns a 256-token slice of the sequence
    for the dense projection, which then consumes gathered ctx^T chunks as
    stationary operands and full w_dense as moving operand, producing final
    [t, o] rows directly (no output transpose; host just concatenates).
"""

import sys
import types

import numpy as np
import ml_dtypes

B, SQ, HIDDEN = 2, 2048, 2048
HEADS, GROUPS, KVC = 16, 4, 128
SCALE = KVC ** -0.5
NCORES = 8
T = B * SQ               # 4096 flattened tokens
TC = 512                 # t-chunk for QKV projection
NTC = T // TC            # 8
NQT = SQ // 128          # 16 q-tiles per batch
MASK_VAL = -1e30


def _install_ntff_hook():
    """boot() skips NTFF hook registration when the image's antenv lacks
    axon_hooks; recreate the tiny module so trace=True / BASS_TRACE works."""
    if "antenv.axon_hooks" in sys.modules:
        return
    try:
        from trn_agent_boot.trn_boot import _ntff_profile_via_ctypes
        hook = _ntff_profile_via_ctypes("/opt/axon/libaxon_pjrt.so")
    except Exception:
        return
    mod = types.ModuleType("antenv.axon_hooks")
    mod.get_axon_ntff_profile_hook = lambda: hook
    mod.set_axon_ntff_profile_hook = lambda h: None
    sys.modules["antenv.axon_hooks"] = mod


_install_ntff_hook()

_CACHE = {}


def _build():
    import concourse.bass as bass
    import concourse.mybir as mybir
    import concourse.tile as tile
    from concourse import bacc
    from concourse.bass import ts, ds

    BF16 = mybir.dt.bfloat16
    F32 = mybir.dt.float32
    AF = mybir.ActivationFunctionType

    nc = bacc.Bacc("TRN2", target_bir_lowering=False, debug=False,
                   num_devices=NCORES)

    # weights come host-pre-tiled to the SBUF layout [p, hc*e] so DMA runs
    # are whole partition rows (multi-KB contiguous)
    xt = nc.dram_tensor("xt", [HIDDEN, T], BF16, kind="ExternalInput")
    wq = nc.dram_tensor("wq", [128, 16 * 256], BF16, kind="ExternalInput")
    # per-core HALF of the kv projection: even cores get w_k, odd cores w_v;
    # the pair exchanges results via a 2-rank AllGather (SPMD-uniform graph,
    # the K/V split lives in the DATA)
    wkv = nc.dram_tensor("wkv", [128, 16 * 128], BF16, kind="ExternalInput")
    wd = nc.dram_tensor("wd", [128, 16 * HIDDEN], BF16, kind="ExternalInput")
    out = nc.dram_tensor("out", [512, HIDDEN], F32, kind="ExternalOutput")

    with tile.TileContext(nc) as tc:
        import contextlib
        with contextlib.ExitStack() as ctx:
            # ---- long-lived pools ----
            persist = ctx.enter_context(tc.tile_pool(name="persist", bufs=1))
            dram = ctx.enter_context(tc.tile_pool(name="dram", bufs=1, space="DRAM"))

            ones_col = persist.tile([128, 1], F32, name="ones_col")
            nc.vector.memset(ones_col[:], 1.0)
            ones_col_bf = persist.tile([128, 1], BF16, name="ones_col_bf")
            nc.vector.memset(ones_col_bf[:], 1.0)
            ones_row = persist.tile([1, 128], F32, name="ones_row")
            nc.vector.memset(ones_row[:], 1.0)

            # weight SBUF tiles: [p, h-chunk, e]
            wq_sb = persist.tile([128, 16, 256], BF16, name="wq_sb")
            nc.sync.dma_start(out=wq_sb[:], in_=wq.ap())
            wkv_sb = persist.tile([128, 16, 128], BF16, name="wkv_sb")
            nc.sync.dma_start(out=wkv_sb[:], in_=wkv.ap())

            # resident Q^T / K^T / V per batch (bf16)
            q_res = [[persist.tile([128, SQ], BF16, name=f"q{h}{b}")
                      for b in range(B)] for h in range(2)]
            k_res = [persist.tile([128, SQ], BF16, name=f"k{b}") for b in range(B)]
            v_res = [persist.tile([128, 16, 128], BF16, name=f"v{b}") for b in range(B)]

            # A2A bounce buffers, one per (b, h_local) chunk
            cc_in = [[dram.tile([NCORES * 128, 256], BF16, name=f"ccin{b}{h}")
                      for h in range(2)] for b in range(B)]
            cc_out = [[dram.tile([NCORES, 128, 256], BF16, name=f"ccout{b}{h}")
                       for h in range(2)] for b in range(B)]
            # pair K/V exchange bounce buffers, per batch
            kv_in = [dram.tile([128, SQ], BF16, name=f"kvin{b}") for b in range(B)]
            kv_out = [dram.tile([2, 128, SQ], BF16, name=f"kvout{b}")
                      for b in range(B)]
            # row-sum DRAM bounce for partition-broadcast (DRAM APs allow the
            # step-0 read that SBUF APs forbid)
            rs_dram = dram.tile([16, 512], mybir.dt.float32, name="rs_dram")



            # ---- phase 1: QKV projections (x^T fully resident, 8KB DMA runs) ----
            with tc.tile_pool(name="p1", bufs=2) as p1, \
                 tc.tile_pool(name="p1x", bufs=1) as p1x, \
                 tc.tile_pool(name="p1ps", bufs=2, space="PSUM") as p1ps:
                # load batch-0 halves of every h-chunk first so chunk 0-3
                # matmuls start after ~half the x^T bytes have landed
                xt_sb = []
                for hc in range(16):
                    t_ = p1x.tile([128, T], BF16, name=f"xt{hc}")
                    nc.sync.dma_start(out=t_[:, 0:SQ],
                                      in_=xt.ap()[ts(hc, 128), 0:SQ])
                    xt_sb.append(t_)
                for hc in range(16):
                    nc.sync.dma_start(out=xt_sb[hc][:, SQ:T],
                                      in_=xt.ap()[ts(hc, 128), SQ:T])
                for tci in range(NTC):
                    b, sc = tci // 4, tci % 4
                    outs = []
                    for name, nbufs, w_ap in (("kv", 2, wkv_sb[:, :, :]),
                                              ("q0", 3, wq_sb[:, :, 0:128]),
                                              ("q1", 3, wq_sb[:, :, 128:256])):
                        ps = p1ps.tile([128, TC], F32, tag=name, bufs=nbufs)
                        for hc in range(16):
                            nc.tensor.matmul(ps[:], w_ap[:, hc, :],
                                             xt_sb[hc][:, ts(tci, TC)],
                                             start=(hc == 0), stop=(hc == 15))
                        outs.append(ps)
                    # my half of K^T/V^T goes to the pair-exchange bounce
                    kvt_sb = p1.tile([128, TC], BF16, tag="kvt")
                    nc.vector.tensor_copy(kvt_sb[:], outs[0][:])
                    nc.sync.dma_start(out=kv_in[b][:, ts(sc, TC)],
                                      in_=kvt_sb[:])
                    nc.vector.tensor_copy(q_res[0][b][:, ts(sc, TC)], outs[1][:])
                    nc.vector.tensor_copy(q_res[1][b][:, ts(sc, TC)], outs[2][:])
                    if sc == 3:
                        # all 4 chunks of this batch staged: exchange K/V
                        # within the core pair (rank 0 of each pair = K)
                        nc.gpsimd.collective_compute(
                            "AllGather", mybir.AluOpType.bypass,
                            replica_groups=[[2 * i, 2 * i + 1]
                                            for i in range(NCORES // 2)],
                            ins=[kv_in[b].opt()],
                            outs=[kv_out[b].opt()])
                        nc.sync.dma_start(out=k_res[b][:],
                                          in_=kv_out[b][0, :, :])
                        for s4 in range(4):
                            nc.sync.dma_start(
                                out=v_res[b][:, ds(4 * s4, 4), :],
                                in_=kv_out[b][1, :, ts(s4, TC)],
                                transpose=True)

            # dense weights pool opens after x^T pool closes (reuses its SBUF);
            # load overlaps attention
            wdp = ctx.enter_context(tc.tile_pool(name="wdp", bufs=1))
            wd_sb = wdp.tile([128, 16, HIDDEN], BF16, name="wd_sb")
            nc.sync.dma_start(out=wd_sb[:], in_=wd.ap())
            # dense gather tiles: loaded during attention as each A2A lands
            p3g = ctx.enter_context(tc.tile_pool(name="p3g", bufs=1))
            g_all = [[p3g.tile([128, NCORES, 256], BF16, name=f"g{b}{h}")
                      for h in range(2)] for b in range(B)]

            # ---- phase 2: attention per (b, h_local), scores^T [k, q] ----
            with tc.tile_pool(name="p2", bufs=2) as p2, \
                 tc.tile_pool(name="p2s", bufs=4) as p2s, \
                 tc.tile_pool(name="p2ps", bufs=2, space="PSUM") as p2ps, \
                 tc.tile_pool(name="p2sc", bufs=4, space="PSUM") as p2sc:
                for b in range(B):
                    for hl in range(2):
                        for qg in range(4):
                            nkt = 4 * (qg + 1)    # causal 128-wide k-tiles
                            et_slab = p2.tile([128, 16, 512], BF16, tag="et",
                                              bufs=3)
                            # QK^T -> exp -> causal zeroing, writes E^T[k, q].
                            # Diagonal k-tiles only compute the valid q-suffix
                            # [128r:512); the prefix is memset to zero.
                            for kt in range(nkt):
                                r = kt - 4 * qg
                                off = 128 * r if r > 0 else 0
                                w = 512 - off
                                sc_ps = p2sc.tile([128, 512], F32, tag="sc")
                                nc.tensor.matmul(
                                    sc_ps[:, 0:w],
                                    k_res[b][:, ts(kt, 128)],
                                    q_res[hl][b][:, ds(qg * 512 + off, w)],
                                    start=True, stop=True)
                                if off:
                                    nc.gpsimd.memset(et_slab[:, kt, 0:off], 0.0)
                                nc.scalar.activation(
                                    et_slab[:, kt, off:512], sc_ps[:, 0:w],
                                    AF.Exp)
                                if r >= 0:
                                    # zero E^T where q < k: keep when f' >= p
                                    nc.gpsimd.affine_select(
                                        out=et_slab[:, kt, off:512],
                                        in_=et_slab[:, kt, off:512],
                                        compare_op=mybir.AluOpType.is_ge,
                                        fill=0.0, base=0,
                                        pattern=[[1, w]],
                                        channel_multiplier=-1)
                            # row sums: ~1/4 of k-tiles folded on PE via
                            # ones-matmuls, the rest on DVE into padd; the f32
                            # matmul folds padd's partitions into the same PSUM
                            # diag k-tiles have a causally-zero prefix of
                            # 128*r columns - narrow the folds past it
                            def _off(kt):
                                r = kt - 4 * qg
                                return 128 * r if r > 0 else 0

                            # PV emitted FIRST: its deps (et tiles) land early,
                            # so the in-order PE never stalls on the row-sum
                            # DVE chain before doing ready PV work
                            ctx_ps = p2ps.tile([128, 512], F32, tag="ctx")
                            for kt in range(nkt):
                                o = _off(kt) if kt > 0 else 0
                                nc.tensor.matmul(ctx_ps[:, o:512],
                                                 v_res[b][:, kt, :],
                                                 et_slab[:, kt, o:512],
                                                 start=(kt == 0), stop=(kt == nkt - 1),
                                                 skip_group_check=True)
                            pe_kts = [kt for kt in range(nkt) if kt % 8 == 7]
                            gps_kts = []
                            dve_kts = [kt for kt in range(nkt) if kt % 8 != 7]
                            rs_ps = p2ps.tile([1, 512], F32, tag="rs", bufs=1)
                            for j, kt in enumerate(pe_kts):
                                o = _off(kt) if j > 0 else 0
                                nc.tensor.matmul(rs_ps[:, o:512], ones_col_bf[:],
                                                 et_slab[:, kt, o:512],
                                                 start=(j == 0), stop=False,
                                                 skip_group_check=True)
                            padd = p2s.tile([128, 512], F32, tag="padd", bufs=2)
                            nc.vector.tensor_add(padd[:], et_slab[:, dve_kts[0], :],
                                                 et_slab[:, dve_kts[1], :])
                            for kt in dve_kts[2:]:
                                o = _off(kt)
                                nc.vector.tensor_add(padd[:, o:512],
                                                     padd[:, o:512],
                                                     et_slab[:, kt, o:512])
                            if gps_kts:
                                # second partial chain on the lightly-loaded
                                # GpSimd engine
                                padd_g = p2s.tile([128, 512], F32, tag="paddg",
                                                  bufs=2)
                                nc.gpsimd.tensor_add(padd_g[:],
                                                     et_slab[:, gps_kts[0], :],
                                                     et_slab[:, gps_kts[1], :])
                                for kt in gps_kts[2:]:
                                    o = _off(kt)
                                    nc.gpsimd.tensor_add(padd_g[:, o:512],
                                                         padd_g[:, o:512],
                                                         et_slab[:, kt, o:512])
                                nc.tensor.matmul(rs_ps[:], ones_col[:], padd_g[:],
                                                 start=False, stop=False,
                                                 skip_group_check=True)
                            nc.tensor.matmul(rs_ps[:], ones_col[:], padd[:],
                                             start=(len(pe_kts) == 0), stop=True,
                                             skip_group_check=True)
                            # rowsums -> SBUF, PE partition-broadcast, recip
                            rs_sb = p2s.tile([1, 512], F32, tag="rssb")
                            nc.vector.tensor_copy(rs_sb[:], rs_ps[:])
                            bc_ps = p2ps.tile([128, 512], F32, tag="bc", bufs=1)
                            nc.tensor.matmul(bc_ps[:], ones_row[:], rs_sb[:],
                                             start=True, stop=True)
                            rinv_sb = p2s.tile([128, 512], F32, tag="rinv", bufs=2)
                            nc.vector.reciprocal_approx_fast(rinv_sb[:], bc_ps[:])
                            # normalized ctx^T -> bf16
                            ctxt_sb = p2.tile([128, 512], BF16, tag="ctxt")
                            nc.vector.tensor_mul(ctxt_sb[:], ctx_ps[:], rinv_sb[:])
                            # scatter to A2A bounce: q-group qg covers peers 2qg, 2qg+1
                            for half in range(2):
                                peer = 2 * qg + half
                                nc.sync.dma_start(
                                    out=cc_in[b][hl][ts(peer, 128), :],
                                    in_=ctxt_sb[:, ts(half, 256)])
                        nc.gpsimd.collective_compute(
                            "AllToAll", mybir.AluOpType.bypass,
                            replica_groups=[list(range(NCORES))],
                            ins=[cc_in[b][hl].opt()],
                            outs=[cc_out[b][hl].opt()])
                        nc.sync.dma_start(
                            out=g_all[b][hl][:],
                            in_=cc_out[b][hl].rearrange("i p s -> p i s"))

            # ---- phase 3: dense projection on my 256-token slice per batch ----
            with tc.tile_pool(name="p3", bufs=2) as p3, \
                 tc.tile_pool(name="p3ps", bufs=2, space="PSUM") as p3ps:
                for b in range(B):
                    for u in range(2):
                        o_ps = p3ps.tile([128, HIDDEN], F32, tag="ops")
                        o_sb = p3.tile([128, HIDDEN], F32, tag="osb")
                        for oc in range(4):
                            for ec in range(16):
                                i, hl = ec // 2, ec % 2
                                nc.tensor.matmul(
                                    o_ps[:, ts(oc, 512)],
                                    g_all[b][hl][:, i, ts(u, 128)],
                                    wd_sb[:, 2 * i + hl, ts(oc, 512)],
                                    start=(ec == 0), stop=(ec == 15))
                            # per-oc epilogue: copy+store overlap the
                            # remaining oc groups' matmuls
                            nc.vector.tensor_copy(o_sb[:, ts(oc, 512)],
                                                  o_ps[:, ts(oc, 512)])
                            nc.sync.dma_start(
                                out=out.ap()[ds(b * 256 + u * 128, 128),
                                             ts(oc, 512)],
                                in_=o_sb[:, ts(oc, 512)])

    nc.compile()
    return nc


def kernel(x, w_q, w_kv, w_dense):
    from concourse.bass_utils import run_bass_kernel_spmd

    bf16 = ml_dtypes.bfloat16
    x = np.asarray(x, dtype=np.float32)
    w_q = np.asarray(w_q, dtype=np.float32)
    w_kv = np.asarray(w_kv, dtype=np.float32)
    w_dense = np.asarray(w_dense, dtype=np.float32)

    xt = np.ascontiguousarray(x.reshape(T, HIDDEN).T).astype(bf16)
    wq_s = (w_q * SCALE).astype(bf16)          # fold softmax scale into Q proj
    wkv_b = w_kv.astype(bf16)
    wd_b = w_dense.astype(bf16)

    def pretile(w):
        # [2048, e] -> SBUF layout [p, hc*e]: row p, col hc*e_sz + e
        e_sz = w.shape[1]
        return np.ascontiguousarray(
            w.reshape(16, 128, e_sz).transpose(1, 0, 2).reshape(128, 16 * e_sz))

    wd_t = pretile(wd_b)
    in_maps = []
    for c in range(NCORES):
        g = c // 2
        if c % 2 == 0:
            wkv_c = wkv_b[:, 128 * g:128 * (g + 1)]                # K half
        else:
            wkv_c = wkv_b[:, 512 + 128 * g:512 + 128 * (g + 1)]    # V half
        in_maps.append({
            "xt": xt,
            "wq": pretile(wq_s[:, 256 * c:256 * (c + 1)]),
            "wkv": pretile(wkv_c),
            "wd": wd_t,
        })

    if "nc" not in _CACHE:
        _CACHE["nc"] = _build()
    nc = _CACHE["nc"]

    res = run_bass_kernel_spmd(nc, in_maps, core_ids=list(range(NCORES)))
    kernel.last_results = res
    kernel.last_exec_time_ns = res.exec_time_ns

    out_full = np.empty((T, HIDDEN), dtype=np.float32)
    for c in range(NCORES):
        r = res.results[c]["out"]              # [512, 2048]
        for b in range(B):
            out_full[b * SQ + 256 * c: b * SQ + 256 * (c + 1), :] = \
                r[b * 256:(b + 1) * 256, :]
    return out_full.reshape(B, SQ, HIDDEN)
